# revision 1
# baseline (speedup 1.0000x reference)
import functools

import jax
import jax.numpy as jnp
import numpy as np

# Baseline factored routing + four deltas, keeping baseline einsum
# layouts (which the Neuron backend lowers well):
#  1. ones-channel fold: i 8->9 with xg1[...,8,:]=1 and Wt[...,8]=bias,
#     so h / cs / bias einsums and their broadcast-adds disappear.
#  2. no max-shift in softmax (|L| = O(1), exp cannot overflow).
#  3. bf16 operands on the big einsums, fp32 accumulation where it
#     matters.
#  4. softmax normalization folded past the y-contraction: y uses
#     unnormalized exp(L) and s_r is divided by the (b,j)-scalar
#     denominator, so c = e/denom is never materialized at (b,j,s,p)
#     size.

NUM_SHARED = 32
IN_DIM = 8
NUM_OUT = 10
OUT_DIM = 16
ROUTE_NUM = 3
EPS = 1e-20

N_CORES = 8
BS = 256
H = 6
P = H * H
I = NUM_SHARED * P

F32 = jnp.float32
BF16 = jnp.bfloat16


def _squash(s):
    n2 = jnp.sum(s * s, axis=2, keepdims=True)
    n = jnp.sqrt(n2)
    return s * (n2 / (1.0 + n2) / (n + EPS))


def _caps_shard(x, w, b_conv):
    bs = x.shape[0]
    S, J, D, Di = NUM_SHARED, NUM_OUT, OUT_DIM, IN_DIM
    xg = x.reshape(bs, S, Di, P)
    ones = jnp.ones((bs, S, 1, P), x.dtype)
    xg1 = jnp.concatenate([xg, ones], axis=2).astype(BF16)   # (b, s, 9, p)

    Wr = w.reshape(S, J, D, Di)
    Br = b_conv.reshape(S, J, D)
    Wt = jnp.concatenate([Wr, Br[..., None]], axis=3).astype(BF16)  # (s, j, d, 9)

    # r0: uniform c
    xs0 = jnp.sum(xg1, axis=3, dtype=F32).astype(BF16)       # (b, s, 9)
    s0 = jnp.einsum('bsi,sjdi->bjd', xs0, Wt, preferred_element_type=F32)
    v = _squash(s0 * (1.0 / I))

    L = None
    for r in range(1, ROUTE_NUM):
        g1 = jnp.einsum('bjd,sjdi->bjsi', v.astype(BF16), Wt,
                        preferred_element_type=BF16)          # (b, j, s, 9)
        dL = jnp.einsum('bjsi,bsip->bjsp', g1, xg1,
                        preferred_element_type=F32)           # (b, j, s, p)
        L = dL if L is None else L + dL

        e = jnp.exp(L)                                        # (b, j, s, p)
        denom = jnp.sum(e.reshape(bs, J, I), axis=2)          # (b, j)
        y1 = jnp.einsum('bjsp,bsip->bjsi', e.astype(BF16), xg1,
                        preferred_element_type=BF16)          # unnormalized
        s_r = jnp.einsum('bjsi,sjdi->bjd', y1, Wt,
                         preferred_element_type=F32) / denom[..., None]
        v = _squash(s_r)
    return v.astype(F32)


@functools.cache
def _pmapped(n_cores: int):
    return jax.pmap(_caps_shard, axis_name='cores', devices=jax.devices()[:n_cores])


def kernel(x: np.ndarray, w: np.ndarray, b_conv: np.ndarray) -> np.ndarray:
    bs = x.shape[0]
    n_cores = N_CORES
    n_dev = len(jax.devices())
    while n_cores > 1 and (n_cores > n_dev or bs % n_cores != 0):
        n_cores //= 2
    shard = bs // n_cores
    xs = np.ascontiguousarray(x.reshape(n_cores, shard, *x.shape[1:]))
    ws = np.ascontiguousarray(np.broadcast_to(w, (n_cores,) + w.shape))
    bs_ = np.ascontiguousarray(np.broadcast_to(b_conv, (n_cores,) + b_conv.shape))
    v = _pmapped(n_cores)(xs, ws, bs_)
    v = np.asarray(v)
    return v.reshape(bs, NUM_OUT, OUT_DIM)



# revision 2
# speedup vs baseline: 9.2637x; 9.2637x over previous
import functools

import jax
import jax.numpy as jnp
import numpy as np

# Factored dynamic routing for the grouped-conv capsule layer, data-parallel
# over 8 NeuronCores (batch 256 -> 32 per core). Deltas vs the earlier
# checkpoint kernel:
#  1. analytic ones-channel: the conv bias is never folded into a 9th input
#     channel, so the (b,s,9,p) concat is never materialized; the bias terms
#     enter as closed-form (b,j,s) broadcasts computed from tiny f32 einsums
#     (this also removes ~11% of the MAC volume of the two big batched
#     contractions).
#  2. bias/ones contributions kept in f32 (rel err 9.3e-4 vs 4.4e-3 for the
#     all-bf16 fold), bf16 only on the large batched einsum operands.
#  3. no max-shift in softmax (|L| = O(1), exp cannot overflow).
#  4. softmax normalization folded past the y-contraction: y uses
#     unnormalized exp(L) and s_r is divided by the (b,j)-scalar denominator,
#     so c = e/denom is never materialized at (b,j,s,p) size.
#  5. e.sum over p (needed for the denominator anyway) doubles as the
#     ones-channel column of the y-contraction, so s_r's bias term reuses it.

NUM_SHARED = 32
IN_DIM = 8
NUM_OUT = 10
OUT_DIM = 16
ROUTE_NUM = 3
EPS = 1e-20

N_CORES = 8
BS = 256
H = 6
P = H * H
I = NUM_SHARED * P

F32 = jnp.float32
BF16 = jnp.bfloat16


def _squash(s):
    n2 = jnp.sum(s * s, axis=2, keepdims=True)
    n = jnp.sqrt(n2)
    return s * (n2 / (1.0 + n2) / (n + EPS))


def _caps_shard(x, w, b_conv):
    bs = x.shape[0]
    S, J, D, Di = NUM_SHARED, NUM_OUT, OUT_DIM, IN_DIM
    xg = x.reshape(bs, S, Di, P).astype(BF16)                # (b,s,8,p)
    Wr = w.reshape(S, J, D, Di).astype(BF16)                 # (s,j,d,8)
    Br = b_conv.reshape(S, J, D)                             # (s,j,d) f32

    # r0: uniform routing coefficients 1/I
    xs0 = jnp.sum(xg, axis=3, dtype=F32).astype(BF16)        # (b,s,8)
    s0 = jnp.einsum('bsi,sjdi->bjd', xs0, Wr, preferred_element_type=F32)
    s0 = s0 + P * jnp.sum(Br, axis=0)[None]                  # ones-channel part
    v = _squash(s0 * (1.0 / I))

    L = None
    for r in range(1, ROUTE_NUM):
        vb = v.astype(BF16)
        g1 = jnp.einsum('bjd,sjdi->bjsi', vb, Wr,
                        preferred_element_type=BF16)         # (b,j,s,8)
        gone = jnp.einsum('bjd,sjd->bjs', v, Br)             # (b,j,s) f32
        dL = jnp.einsum('bjsi,bsip->bjsp', g1, xg,
                        preferred_element_type=F32)          # (b,j,s,p)
        dL = dL + gone[..., None]
        L = dL if L is None else L + dL

        e = jnp.exp(L)                                       # (b,j,s,p)
        es = jnp.sum(e, axis=3)                              # (b,j,s)
        denom = jnp.sum(es, axis=2)                          # (b,j)
        y1 = jnp.einsum('bjsp,bsip->bjsi', e.astype(BF16), xg,
                        preferred_element_type=BF16)         # unnormalized
        s_r = jnp.einsum('bjsi,sjdi->bjd', y1, Wr, preferred_element_type=F32)
        s_r = s_r + jnp.einsum('bjs,sjd->bjd', es, Br)
        s_r = s_r / denom[..., None]
        v = _squash(s_r)
    return v.astype(F32)


@functools.cache
def _pmapped(n_cores: int):
    return jax.pmap(_caps_shard, axis_name='cores', devices=jax.devices()[:n_cores])


def kernel(x: np.ndarray, w: np.ndarray, b_conv: np.ndarray) -> np.ndarray:
    bs = x.shape[0]
    n_cores = N_CORES
    n_dev = len(jax.devices())
    while n_cores > 1 and (n_cores > n_dev or bs % n_cores != 0):
        n_cores //= 2
    shard = bs // n_cores
    xs = np.ascontiguousarray(x.reshape(n_cores, shard, *x.shape[1:]))
    ws = np.ascontiguousarray(np.broadcast_to(w, (n_cores,) + w.shape))
    bs_ = np.ascontiguousarray(np.broadcast_to(b_conv, (n_cores,) + b_conv.shape))
    v = _pmapped(n_cores)(xs, ws, bs_)
    v = np.asarray(v)
    return v.reshape(bs, NUM_OUT, OUT_DIM)


# revision 5
# speedup vs baseline: 10.8330x; 1.1694x over previous
import functools

import jax
import jax.numpy as jnp
import numpy as np

# Factored dynamic routing for the grouped-conv capsule layer, data-parallel
# over 8 NeuronCores (batch 256 -> 32 per core). Device time is dominated by
# per-instruction overhead inside the NEFF (~15-30 us/op regardless of tensor
# size), so the device body is op-minimal:
#  1. ALL input prep on host: ones-plane concat, bias column fold, bf16
#     casts. The device receives ready-to-use xg1 (b,s,9,p) bf16 and
#     wt (s,j,d,9) bf16 and runs only the 5-einsum routing loop.
#  2. logits and exp in bf16 end-to-end; f32 only for reductions and the
#     final s_r scaling.
#  3. squash factor n/(1+n^2), algebraically equal to the reference's
#     n2/(1+n2)/(n+EPS) up to O(EPS).
#  4. no max-shift in softmax (|L| = O(1), exp cannot overflow); softmax
#     normalization folded past the y-contraction (c = e/denom never
#     materialized).

NUM_SHARED = 32
IN_DIM = 8
NUM_OUT = 10
OUT_DIM = 16
ROUTE_NUM = 3
EPS = 1e-20

N_CORES = 8
BS = 256
H = 6
P = H * H
I = NUM_SHARED * P

F32 = jnp.float32
BF16 = jnp.bfloat16


def _squash(s):
    n2 = jnp.sum(s * s, axis=2, keepdims=True)
    return s * (jnp.sqrt(n2) / (1.0 + n2))


def _caps_shard(xg1, wt):
    # xg1: (bs, S, 9, P) bf16 with ones plane at i=8
    # wt:  (S, J, D, 9) bf16 with bias column at i=8
    bs = xg1.shape[0]
    J = NUM_OUT

    xs0 = jnp.sum(xg1, axis=3, dtype=F32).astype(BF16)       # (b,s,9)
    s0 = jnp.einsum('bsi,sjdi->bjd', xs0, wt, preferred_element_type=F32)
    v = _squash(s0 * (1.0 / I))

    L = None
    for r in range(1, ROUTE_NUM):
        g1 = jnp.einsum('bjd,sjdi->bjsi', v.astype(BF16), wt,
                        preferred_element_type=BF16)         # (b,j,s,9)
        dL = jnp.einsum('bjsi,bsip->bjsp', g1, xg1,
                        preferred_element_type=BF16)         # (b,j,s,p)
        L = dL if L is None else L + dL
        e = jnp.exp(L)                                       # bf16
        denom = jnp.sum(e.reshape(bs, J, I), axis=2, dtype=F32)
        y1 = jnp.einsum('bjsp,bsip->bjsi', e, xg1,
                        preferred_element_type=BF16)         # unnormalized
        s_r = jnp.einsum('bjsi,sjdi->bjd', y1, wt,
                         preferred_element_type=F32) / denom[..., None]
        v = _squash(s_r)
    return v.astype(F32)


@functools.cache
def _pmapped(n_cores: int):
    return jax.pmap(_caps_shard, axis_name='cores', devices=jax.devices()[:n_cores])


def _prep_host(x, w, b_conv):
    bs = x.shape[0]
    S, J, D, Di = NUM_SHARED, NUM_OUT, OUT_DIM, IN_DIM
    xg = np.asarray(x, np.float32).reshape(bs, S, Di, P)
    xg1 = np.concatenate([xg, np.ones((bs, S, 1, P), np.float32)], axis=2)
    wt = np.concatenate(
        [np.asarray(w, np.float32).reshape(S, J, D, Di),
         np.asarray(b_conv, np.float32).reshape(S, J, D)[..., None]], axis=3)
    return xg1.astype(BF16), wt.astype(BF16)


def kernel(x: np.ndarray, w: np.ndarray, b_conv: np.ndarray) -> np.ndarray:
    bs = x.shape[0]
    n_cores = N_CORES
    n_dev = len(jax.devices())
    while n_cores > 1 and (n_cores > n_dev or bs % n_cores != 0):
        n_cores //= 2
    shard = bs // n_cores
    xg1, wt = _prep_host(x, w, b_conv)
    xs = np.ascontiguousarray(xg1.reshape(n_cores, shard, *xg1.shape[1:]))
    ws = np.ascontiguousarray(np.broadcast_to(wt, (n_cores,) + wt.shape))
    v = _pmapped(n_cores)(xs, ws)
    v = np.asarray(v)
    return v.reshape(bs, NUM_OUT, OUT_DIM)
